# revision 6
# baseline (speedup 1.0000x reference)
"""Adaptive polyphase sampling (stride 2, p=2) on 8 TRN2 NeuronCores.

For x [32, 256, 64, 64] f32: compute the 4 polyphase components
x[:, :, i::2, j::2], pick per-sample the component with the largest L2
norm (over channels+space), return it [32, 256, 32, 32].

Sharding: pure data parallel over batch — 4 samples per core, no
cross-core communication.

Per-core dataflow (samples s = 0..3, channel halves hh = 0..1):
  sync   : DMA x[s] -> samp[s%3]                       (4 MiB each)
  scalar : norms k=0..3 (Square activation + accum_out) -> norms[:, 4s+k]
           selection t0 = V_0 * mask_0                 (Copy act, scale AP)
  vector : mask: reduce_max(psum) + is_equal           -> mask[:, 4s:4s+4]
           selection t_k for k=1,2,3, adds b,a,O       -> obuf[s%2]
  tensor : ones[128,128] @ norms[:, 4s:4s+4] -> psum   (channel reduce
           + broadcast of per-sample component norms to all partitions)
  gpsimd : memset ones; DMA obuf -> out[s]             (1 MiB each)

The argmax is realized as mask_k = (norm_k == max_k norm_k) in {0,1},
then O = sum_k mask_k * V_k. Exact float ties between component norms
(sums of ~1M random squares) are probability-zero.

Synchronization: TRN2 engines are pipelined, so even same-engine data
deps need semaphore handshakes (one op's writes only land at its
DRAIN). Every scalar/vector op increments its engine chain sem (sch /
vch) and waits for all previously-emitted ops on that engine;
cross-engine waits reference chain-sem thresholds computed from the
static emission plan below. DMA completions are unordered across
transactions, so each in-flight DMA gets its own semaphore.
"""

from contextlib import ExitStack

import numpy as np

import concourse.bass as bass
from concourse import mybir
from concourse.bass_utils import run_bass_kernel_spmd

F32 = mybir.dt.float32
AX = mybir.AxisListType
OP = mybir.AluOpType
ACT = mybir.ActivationFunctionType

B, C, H, W = 32, 256, 64, 64
NCORES = 8
SPC = B // NCORES          # samples per core
H2, W2 = H // 2, W // 2    # 32, 32
SP = H * W                 # 4096 spatial elems per (sample, half)
OSP = H2 * W2              # 1024

N_SAMP_BUFS = 3
N_OBUFS = 2

# ---- static emission plan (op counts per group) --------------------------
SC_SIZES = {"n": 4, "sel": 1}
VE_SIZES = {"mk": 2, "vs": 6}
SC_ORDER = [("n", 0), ("n", 1), ("sel", 0), ("n", 2), ("sel", 1),
            ("n", 3), ("sel", 2), ("sel", 3)]
VE_ORDER = [("mk", 0), ("vs", 0), ("mk", 1), ("vs", 1),
            ("mk", 2), ("vs", 2), ("mk", 3), ("vs", 3)]


def _plan(order, sizes):
    done, start, c = {}, {}, 0
    for g, s in order:
        start[(g, s)] = c
        c += sizes[g]
        done[(g, s)] = c
    return done, start, c


SC_DONE, SC_START, SC_TOTAL = _plan(SC_ORDER, SC_SIZES)
VE_DONE, VE_START, VE_TOTAL = _plan(VE_ORDER, VE_SIZES)
# a-add (consumes t0/t1) is the 5th op inside a "vs" block
VE_A_DONE = {s: VE_START[("vs", s)] + 5 for s in range(SPC)}


def _comp_view(samp_ap, k):
    """[128, 2, 32, 32] strided view of polyphase component k=(i,j)."""
    i, j = divmod(k, 2)
    v6 = samp_ap.rearrange("p hh (r i c j) -> p hh r i c j", r=H2, i=2, c=W2, j=2)
    return v6[:, :, :, i, :, j]


def build_nc():
    nc = bass.Bass("TRN2", target_bir_lowering=False, debug=False)
    x = nc.dram_tensor("x", [SPC, C, H, W], F32, kind="ExternalInput")
    out = nc.dram_tensor("out", [SPC, C, H2, W2], F32, kind="ExternalOutput")

    x_aps = [x.ap()[s].rearrange("(hh p) h w -> p hh (h w)", p=128) for s in range(SPC)]
    out_aps = [
        out.ap()[s].rearrange("(hh p) a b -> p hh (a b)", p=128) for s in range(SPC)
    ]

    with ExitStack() as ctx:
        block = ctx.enter_context(nc.Block())
        sem = lambda name: ctx.enter_context(nc.semaphore(name))
        sb = lambda name, shape: ctx.enter_context(nc.sbuf_tensor(name, shape, F32))
        dmains = [sem(f"dmain{s}") for s in range(SPC)]
        dmaouts = [sem(f"dmaout{i}") for i in range(N_OBUFS)]
        g1 = sem("g1")
        sch, vch, mm = sem("sch"), sem("vch"), sem("mm")
        samps = [sb(f"samp{i}", [128, 2, SP]) for i in range(N_SAMP_BUFS)]
        obufs = [sb(f"obuf{i}", [128, 2, OSP]) for i in range(N_OBUFS)]
        tbuf = [sb(f"t{i}", [128, 2, OSP]) for i in range(4)]
        asum = sb("asum", [128, 2, OSP])
        bsum = sb("bsum", [128, 2, OSP])
        sqs = sb("sqs", [128, 2, OSP])
        norms = sb("norms", [128, 4 * SPC])
        mask = sb("mask", [128, 4 * SPC])
        mx = sb("mx", [128, SPC])
        ones = sb("ones", [128, 128])
        psums = [
            ctx.enter_context(nc.psum_tensor(f"ps{i}", [128, 4], F32))
            for i in range(2)
        ]

        def V(s, k):
            return _comp_view(samps[s % N_SAMP_BUFS].ap(), k)

        sq_view = lambda t: t.ap().rearrange("p hh (r c) -> p hh r c", r=H2)
        ncol = lambda s, k: norms.ap()[:, 4 * s + k : 4 * s + k + 1]
        mcol = lambda s, k: mask.ap()[:, 4 * s + k : 4 * s + k + 1]

        @block.sync
        def _(sync):
            for s in range(SPC):
                if s >= N_SAMP_BUFS:
                    # samp[s%3] reuse: all consumers of sample s-3 done
                    sp = s - N_SAMP_BUFS
                    sync.wait_ge(sch, SC_DONE[("sel", sp)])
                    sync.wait_ge(vch, VE_DONE[("vs", sp)])
                sync.dma_start(out=samps[s % N_SAMP_BUFS].ap(), in_=x_aps[s]).then_inc(
                    dmains[s], 16
                )

        @block.gpsimd
        def _(gpsimd):
            gpsimd.memset(ones.ap(), 1.0).then_inc(g1, 1)
            for s in range(SPC):
                gpsimd.wait_ge(vch, VE_DONE[("vs", s)])
                gpsimd.dma_start(out=out_aps[s], in_=obufs[s % N_OBUFS].ap()).then_inc(
                    dmaouts[s % N_OBUFS], 16
                )

        @block.tensor
        def _(tensor):
            tensor.wait_ge(g1, 1)
            for s in range(SPC):
                tensor.wait_ge(sch, SC_DONE[("n", s)])
                if s >= 2:
                    # psum[s%2] reuse: mask ops of sample s-2 done reading
                    tensor.wait_ge(vch, VE_DONE[("mk", s - 2)])
                tensor.matmul(
                    psums[s % 2].ap(),
                    ones.ap(),
                    norms.ap()[:, 4 * s : 4 * s + 4],
                    start=True,
                    stop=True,
                ).then_inc(mm, 1)

        @block.scalar
        def _(scalar):
            cnt = [0]

            def emit(inst):
                inst.then_inc(sch, 1)
                cnt[0] += 1

            def barrier():
                if cnt[0]:
                    scalar.wait_ge(sch, cnt[0])

            def n(s):
                scalar.wait_ge(dmains[s], 16)
                for k in range(4):
                    barrier()
                    emit(
                        scalar.activation(
                            sq_view(sqs), V(s, k), ACT.Square, accum_out=ncol(s, k)
                        )
                    )

            def selp(s):
                scalar.wait_ge(vch, VE_DONE[("mk", s)])
                if s >= 1:
                    # t0 reuse: a-add of sample s-1 consumed it
                    scalar.wait_ge(vch, VE_A_DONE[s - 1])
                barrier()
                emit(
                    scalar.activation(
                        sq_view(tbuf[0]), V(s, 0), ACT.Copy, scale=mcol(s, 0)
                    )
                )

            for g, s in SC_ORDER:
                n(s) if g == "n" else selp(s)
            assert cnt[0] == SC_TOTAL

        @block.vector
        def _(vector):
            cnt = [0]

            def emit(inst):
                inst.then_inc(vch, 1)
                cnt[0] += 1

            def barrier():
                if cnt[0]:
                    vector.wait_ge(vch, cnt[0])

            def mk(s):
                vector.wait_ge(mm, s + 1)
                barrier()
                emit(
                    vector.reduce_max(
                        mx.ap()[:, s : s + 1], psums[s % 2].ap(), axis=AX.X
                    )
                )
                barrier()
                emit(
                    vector.tensor_scalar(
                        out=mask.ap()[:, 4 * s : 4 * s + 4],
                        in0=psums[s % 2].ap(),
                        scalar1=mx.ap()[:, s : s + 1],
                        scalar2=None,
                        op0=OP.is_equal,
                    )
                )

            def vs(s):
                for k in (1, 2, 3):
                    barrier()
                    emit(vector.tensor_scalar_mul(sq_view(tbuf[k]), V(s, k), mcol(s, k)))
                barrier()
                emit(vector.tensor_add(bsum.ap(), tbuf[2].ap(), tbuf[3].ap()))
                vector.wait_ge(sch, SC_DONE[("sel", s)])
                barrier()
                emit(vector.tensor_add(asum.ap(), tbuf[0].ap(), tbuf[1].ap()))
                if s >= N_OBUFS:
                    vector.wait_ge(dmaouts[s % N_OBUFS], 16 * (s // N_OBUFS))
                barrier()
                emit(
                    vector.tensor_add(obufs[s % N_OBUFS].ap(), asum.ap(), bsum.ap())
                )

            fns = {"mk": mk, "vs": vs}
            for g, s in VE_ORDER:
                fns[g](s)
            assert cnt[0] == VE_TOTAL

    return nc


_NC_CACHE = None


def _get_nc():
    global _NC_CACHE
    if _NC_CACHE is None:
        _NC_CACHE = build_nc()
    return _NC_CACHE


def kernel(x) -> np.ndarray:
    x = np.asarray(x, dtype=np.float32)
    assert x.shape == (B, C, H, W), x.shape
    shards = np.split(x, NCORES, axis=0)
    in_maps = [{"x": s} for s in shards]
    res = run_bass_kernel_spmd(_get_nc(), in_maps, core_ids=list(range(NCORES)))
    return np.concatenate([r["out"] for r in res.results], axis=0)
